# revision 30
# baseline (speedup 1.0000x reference)
"""Trainium2 Bass kernel for nn_Decoder (2-layer LSTM + 3 FC + top-k beam decode).

Strategy: pure data parallelism over batch (2048 -> 8 cores x 256).
All activations feature-major [feat, batch]. All matmuls in float32r
(FP22 multiply, fp32 accumulate, 1 cycle/row at free-dim >= 256).
Argmax/top-k via DVE max8 + max_index; embedding gather via one-hot matmul.
Host assembles the [B,16,4,2] trajectory from per-step argmax indices.
"""
import numpy as np

B, D, H = 2048, 256, 256
K4, QW, QL, DELTA = 4, 100, 100, 16
Q = QW * QL
NCORES = 8
BS = B // NCORES          # 256 rows per core
NT = 20                   # fc3 N-tiles of 500
TW = 500

_CACHE = {}


def _build_nc(delta=DELTA, dbg=False):
    import concourse.mybir as mybir
    import concourse.tile as tile
    import concourse.bacc as bacc
    from concourse.masks import make_identity

    F32 = mybir.dt.float32
    U32 = mybir.dt.uint32
    AF = mybir.ActivationFunctionType
    ALU = mybir.AluOpType

    nc = bacc.Bacc(None, target_bir_lowering=False, debug=False)

    def din(name, shape, dt=F32):
        return nc.dram_tensor(name, shape, dt, kind="ExternalInput")

    F32R = mybir.dt.float32r
    x_in = din("x_fm", [2, 128, BS], F32R)
    h1_in = din("h1_fm", [2, 128, BS], F32R)
    c1_in = din("c1_fm", [2, 128, BS])
    h2_in = din("h2_fm", [2, 128, BS], F32R)
    c2_in = din("c2_fm", [2, 128, BS])
    w1ih_in = din("w1ihT", [2, 128, 4 * H], F32R)
    w1hh_in = din("w1hhT", [2, 128, 4 * H], F32R)
    w2ih_in = din("w2ihT", [2, 128, 4 * H], F32R)
    w2hh_in = din("w2hhT", [2, 128, 4 * H], F32R)
    fc1_in = din("fc1T", [2, 128, H], F32R)
    fc2_in = din("fc2T", [2, 128, H], F32R)
    fc3_in = din("fc3T", [2, 128, Q], F32R)
    fcqw_in = din("fcqwT", [100, 128], F32R)
    fcql_in = din("fcqlT", [100, 128], F32R)
    b1_in = din("b1r", [128, 8])
    b2_in = din("b2r", [128, 8])
    fc1b_in = din("fc1br", [128, 2])
    fc2b_in = din("fc2br", [128, 2])
    b3rep_in = din("b3rep", [128, Q])
    fcqwb_in = din("fcqwb", [128, 1])
    fcqlb_in = din("fcqlb", [128, 1])

    idx_out = nc.dram_tensor("idx_out", [2, 128, 20], U32, kind="ExternalOutput")

    with tile.TileContext(nc) as tc:
        with (
            tc.tile_pool(name="wp", bufs=1) as wp,
            tc.tile_pool(name="st", bufs=1) as st,
            tc.tile_pool(name="wk", bufs=2) as wk,
            tc.tile_pool(name="ps", bufs=2, space="PSUM") as ps,
        ):
            # ---- load weights / consts ----
            def wload(src, shape, tag, dt=F32):
                t = wp.tile(shape, dt, tag=tag, name=tag)
                if len(shape) == 3 and shape[1] == 2:
                    nc.sync.dma_start(t[:], src[:].rearrange("c p f -> p c f"))
                else:
                    nc.sync.dma_start(t[:], src[:])
                return t

            w1ih = wload(w1ih_in, [128, 2, 4 * H], "w1ih", F32R)
            w1hh = wload(w1hh_in, [128, 2, 4 * H], "w1hh", F32R)
            w2ih = wload(w2ih_in, [128, 2, 4 * H], "w2ih", F32R)
            w2hh = wload(w2hh_in, [128, 2, 4 * H], "w2hh", F32R)
            fc1 = wload(fc1_in, [128, 2, H], "fc1", F32R)
            fc2 = wload(fc2_in, [128, 2, H], "fc2", F32R)
            fc3w = wload(fc3_in, [128, 2, Q], "fc3w", F32R)
            fcqw = wload(fcqw_in, [100, 128], "fcqw", F32R)
            fcql = wload(fcql_in, [100, 128], "fcql", F32R)
            b1r = wload(b1_in, [128, 8], "b1r")
            b2r = wload(b2_in, [128, 8], "b2r")
            fc1b = wload(fc1b_in, [128, 2], "fc1b")
            fc2b = wload(fc2b_in, [128, 2], "fc2b")
            b3rep = wload(b3rep_in, [128, Q], "b3rep")
            fcqwb = wload(fcqwb_in, [128, 1], "fcqwb")
            fcqlb = wload(fcqlb_in, [128, 1], "fcqlb")

            ident = wp.tile([128, 128], F32)
            make_identity(nc, ident[:])
            io_f = wp.tile([128, 100], F32)
            nc.gpsimd.iota(io_f[:], pattern=[[1, 100]], base=0, channel_multiplier=0,
                           allow_small_or_imprecise_dtypes=True)
            io100 = wp.tile([128, 100], F32)
            nc.gpsimd.iota(io100[:], pattern=[[100, 100]], base=0,
                           channel_multiplier=0,
                           allow_small_or_imprecise_dtypes=True)
            io32 = wp.tile([128, 32], F32)
            nc.gpsimd.iota(io32[:], pattern=[[1, 32]], base=0, channel_multiplier=0,
                           allow_small_or_imprecise_dtypes=True)

            # ---- persistent states (feature-major [128, chunk, BS]) ----
            def sload(src, tag, dt=F32):
                t = st.tile([128, 2, BS], dt, tag=tag, name=tag)
                nc.sync.dma_start(t[:], src[:].rearrange("c p b -> p c b"))
                return t

            h1_t = sload(h1_in, "h1", F32R)
            c1_t = sload(c1_in, "c1")
            h2_t = sload(h2_in, "h2", F32R)
            c2_t = sload(c2_in, "c2")
            emb_t = st.tile([128, 2, BS], F32R, tag="emb", name="emb")
            nc.sync.dma_start(emb_t[:], x_in[:].rearrange("c p b -> p c b"))
            outi = st.tile([128, 2, 20], U32, tag="outi", name="outi")
            nc.vector.memset(outi[:], 0)

            def pbig():
                return ps.tile([128, 4, 512], F32, tag="big", name="big")

            def lstm_layer(inp, hT, cT, wih, whh, br):
                gpt = pbig()

                def gsl(g):
                    return gpt[:, g // 2, (g % 2) * 256:(g % 2) * 256 + 256]

                for g in range(8):
                    sl = slice(128 * g, 128 * (g + 1))
                    nc.tensor.matmul(gsl(g), wih[:, 0, sl], inp[:, 0, :],
                                     start=True, stop=False)
                    nc.tensor.matmul(gsl(g), wih[:, 1, sl], inp[:, 1, :],
                                     start=False, stop=False)
                    nc.tensor.matmul(gsl(g), whh[:, 0, sl], hT[:, 0, :],
                                     start=False, stop=False)
                    nc.tensor.matmul(gsl(g), whh[:, 1, sl], hT[:, 1, :],
                                     start=False, stop=True)
                for ch in range(2):
                    si = wk.tile([128, 256], F32, tag="si", bufs=1)
                    sf = wk.tile([128, 256], F32, tag="sf", bufs=1)
                    tg = wk.tile([128, 256], F32, tag="tg", bufs=1)
                    so = wk.tile([128, 256], F32, tag="so", bufs=1)
                    nc.scalar.activation(si[:], gsl(0 + ch), AF.Sigmoid,
                                         bias=br[:, 0 + ch:1 + ch])
                    nc.scalar.activation(sf[:], gsl(2 + ch), AF.Sigmoid,
                                         bias=br[:, 2 + ch:3 + ch])
                    nc.scalar.activation(tg[:], gsl(4 + ch), AF.Tanh,
                                         bias=br[:, 4 + ch:5 + ch])
                    nc.scalar.activation(so[:], gsl(6 + ch), AF.Sigmoid,
                                         bias=br[:, 6 + ch:7 + ch])
                    t1 = wk.tile([128, 256], F32, tag="t1", bufs=1)
                    t2 = wk.tile([128, 256], F32, tag="t2", bufs=1)
                    nc.vector.tensor_mul(t1[:], sf[:], cT[:, ch, :])
                    nc.vector.tensor_mul(t2[:], si[:], tg[:])
                    nc.vector.tensor_add(cT[:, ch, :], t1[:], t2[:])
                    t3 = wk.tile([128, 256], F32, tag="t3", bufs=1)
                    nc.scalar.activation(t3[:], cT[:, ch, :], AF.Tanh)
                    nc.vector.tensor_mul(hT[:, ch, :], so[:], t3[:])

            for t in range(delta):
                inp = emb_t
                lstm_layer(inp, h1_t, c1_t, w1ih, w1hh, b1r)
                lstm_layer(h1_t, h2_t, c2_t, w2ih, w2hh, b2r)

                # fc1, fc2 (feature-major out)
                y1 = st.tile([128, 2, BS], F32R, tag="y1")
                y2 = st.tile([128, 2, BS], F32R, tag="y2")
                for (dst, w, bb, src) in ((y1, fc1, fc1b, h2_t), (y2, fc2, fc2b, y1)):
                    fpt = pbig()
                    for m in range(2):
                        fsl = fpt[:, m // 2, (m % 2) * 256:(m % 2) * 256 + 256]
                        sl = slice(128 * m, 128 * (m + 1))
                        nc.tensor.matmul(fsl, w[:, 0, sl], src[:, 0, :],
                                         start=True, stop=False)
                        nc.tensor.matmul(fsl, w[:, 1, sl], src[:, 1, :],
                                         start=False, stop=True)
                        nc.scalar.activation(dst[:, m, :], fsl, AF.Identity,
                                             bias=bb[:, m:m + 1])

                # fc3 per batch-chunk: 5 groups x 4 tiles of 500
                ohwT = wk.tile([100, 256], F32R, tag="ohwT", name="ohwT")
                ohlT = wk.tile([100, 256], F32R, tag="ohlT", name="ohlT")
                for bc in range(2):
                    bsl = slice(128 * bc, 128 * (bc + 1))
                    lq = [wk.tile([128, 2500], F32, tag="logq", name="logq",
                                  bufs=2) for _ in range(4)]
                    cand_v = wk.tile([128, 32], F32, tag="candv", name="candv")
                    cand_i = wk.tile([128, 32], F32, tag="candi", name="candi")
                    nscan = [0]

                    def scan_ready(upto):
                        # scan any quarter fully evacuated below `upto`
                        while nscan[0] < 4 and (nscan[0] + 1) * 2500 <= upto:
                            qt = nscan[0]
                            m8q = wk.tile([128, 8], F32, tag="m8q", name="m8q")
                            i8q = wk.tile([128, 8], U32, tag="i8q", name="i8q")
                            nc.vector.max(m8q[:], lq[qt][:])
                            nc.vector.max_index(i8q[:], m8q[:], lq[qt][:])
                            nc.vector.tensor_copy(cand_v[:, 8 * qt:8 * qt + 8],
                                                  m8q[:])
                            i8f = wk.tile([128, 8], F32, tag="i8f", name="i8f")
                            nc.vector.tensor_copy(i8f[:], i8q[:])
                            nc.vector.tensor_scalar(
                                cand_i[:, 8 * qt:8 * qt + 8], i8f[:],
                                float(2500 * qt), None, op0=ALU.add)
                            nscan[0] += 1

                    for grp in range(5):
                        gp3 = pbig()
                        for tt in range(4):
                            n0 = (grp * 4 + tt) * TW
                            o = gp3[:, tt, 0:TW]
                            nc.tensor.matmul(o, y2[:, 0, bsl],
                                             fc3w[:, 0, n0:n0 + TW],
                                             start=True, stop=False)
                            nc.tensor.matmul(o, y2[:, 1, bsl],
                                             fc3w[:, 1, n0:n0 + TW],
                                             start=False, stop=True)
                        # evacuate per psum tile (+bias) into quarter tiles
                        for tt in range(4):
                            n0 = (grp * 4 + tt) * TW
                            qt = n0 // 2500
                            nc.vector.tensor_add(
                                lq[qt][:, n0 - 2500 * qt:n0 - 2500 * qt + TW],
                                gp3[:, tt, 0:TW],
                                b3rep[:, n0:n0 + TW])
                        scan_ready(grp * 2000 + 2000)

                    # merge 32 candidates
                    vm8 = wk.tile([128, 8], F32, tag="vm8", name="vm8")
                    pm8 = wk.tile([128, 8], U32, tag="pm8", name="pm8")
                    nc.vector.max(vm8[:], cand_v[:])
                    nc.vector.max_index(pm8[:], vm8[:], cand_v[:])
                    pmf = wk.tile([128, 8], F32, tag="pmf", name="pmf")
                    nc.vector.tensor_copy(pmf[:], pm8[:])
                    nk = 4 if t == 0 else 1
                    qsel = wk.tile([128, 4], F32, tag="qsel", name="qsel")
                    for kk in range(nk):
                        ohp = wk.tile([128, 32], F32, tag="ohp", name="ohp")
                        nc.vector.tensor_scalar(ohp[:], io32[:], pmf[:, kk:kk + 1],
                                                None, op0=ALU.is_equal)
                        tmq = wk.tile([128, 32], F32, tag="tmq", name="tmq")
                        nc.vector.tensor_mul(tmq[:], ohp[:], cand_i[:])
                        nc.vector.tensor_reduce(qsel[:, kk:kk + 1], tmq[:],
                                                axis=mybir.AxisListType.X,
                                                op=ALU.add)
                    if t == 0:
                        nc.vector.tensor_copy(outi[:, bc, 0:4], qsel[:, 0:4])
                    else:
                        nc.vector.tensor_copy(outi[:, bc, 4 + t - 1:5 + t - 1],
                                              qsel[:, 0:1])
                    if t == delta - 1:
                        continue
                    qf = wk.tile([128, 1], F32, tag="qf", name="qf")
                    nc.vector.tensor_copy(qf[:], qsel[:, 0:1])
                    # ohw[b,j] = (100j <= q) & (100j > q-100)
                    m_ge = wk.tile([128, 100], F32, tag="mge", name="mge", bufs=1)
                    nc.vector.tensor_scalar(m_ge[:], io100[:], qf[:], None,
                                            op0=ALU.is_le)
                    qm = wk.tile([128, 1], F32, tag="qm", name="qm")
                    nc.vector.tensor_scalar(qm[:], qf[:], -100.0, None, op0=ALU.add)
                    m_lt = wk.tile([128, 100], F32, tag="mlt", name="mlt", bufs=1)
                    nc.vector.tensor_scalar(m_lt[:], io100[:], qm[:], None,
                                            op0=ALU.is_gt)
                    ohw = wk.tile([128, 100], F32, tag="ohw", name="ohw", bufs=1)
                    nc.vector.tensor_mul(ohw[:], m_ge[:], m_lt[:])
                    tm = wk.tile([128, 100], F32, tag="tm", name="tm", bufs=1)
                    nc.vector.tensor_mul(tm[:], ohw[:], io_f[:])
                    fwf = wk.tile([128, 1], F32, tag="fwf", name="fwf")
                    nc.vector.tensor_reduce(fwf[:], tm[:], axis=mybir.AxisListType.X,
                                            op=ALU.add)
                    flf = wk.tile([128, 1], F32, tag="flf", name="flf")
                    nc.vector.tensor_scalar(flf[:], fwf[:], -100.0, qf[:],
                                            op0=ALU.mult, op1=ALU.add)
                    ohl = wk.tile([128, 100], F32, tag="ohl", name="ohl", bufs=1)
                    nc.vector.tensor_scalar(ohl[:], io_f[:], flf[:], None,
                                            op0=ALU.is_equal)
                    ptr = pbig()
                    pw = ptr[0:100, 0, 0:128]
                    nc.tensor.transpose(pw, ohw[:], ident[:])
                    nc.vector.tensor_copy(ohwT[:, bsl128(bc)], pw)
                    pl = ptr[0:100, 1, 0:128]
                    nc.tensor.transpose(pl, ohl[:], ident[:])
                    nc.vector.tensor_copy(ohlT[:, bsl128(bc)], pl)

                if t == delta - 1:
                    continue
                # embedding gather matmuls + bias
                pet = pbig()
                pe0 = pet[:, 0, 0:BS]
                pe1 = pet[:, 1, 0:BS]
                nc.tensor.matmul(pe0, fcqw[:], ohwT[:], start=True, stop=True)
                nc.tensor.matmul(pe1, fcql[:], ohlT[:], start=True, stop=True)
                nc.scalar.activation(emb_t[:, 0, :], pe0, AF.Identity,
                                     bias=fcqwb[:])
                nc.scalar.activation(emb_t[:, 1, :], pe1, AF.Identity,
                                     bias=fcqlb[:])

            for bc in range(2):
                nc.sync.dma_start(idx_out[bc], outi[:, bc, :])
    nc.finalize()
    return nc


def bsl128(bc):
    return slice(128 * bc, 128 * (bc + 1))


def _prep_shared(inputs):
    f32 = np.float32

    def fm(w):  # [out,in] -> lhsT layout [2,128,out]
        wt = np.ascontiguousarray(w.T.astype(f32))        # [in, out]
        return wt.reshape(2, 128, wt.shape[1])

    fc3T = np.ascontiguousarray(inputs["fc3_W"].T.astype(f32))  # [256, 10000]

    shared = {
        "w1ihT": fm(inputs["lstm1_Wih"]),
        "w1hhT": fm(inputs["lstm1_Whh"]),
        "w2ihT": fm(inputs["lstm2_Wih"]),
        "w2hhT": fm(inputs["lstm2_Whh"]),
        "fc1T": fm(inputs["fc1_W"]),
        "fc2T": fm(inputs["fc2_W"]),
        "fc3T": fc3T.reshape(2, 128, Q),
        "fcqwT": np.ascontiguousarray(inputs["fcqw_W"].T.astype(f32))[:, :],
        "fcqlT": np.ascontiguousarray(inputs["fcql_W"].T.astype(f32))[:, :],
        "b1r": inputs["lstm1_b"].astype(f32).reshape(8, 128).T.copy(),
        "b2r": inputs["lstm2_b"].astype(f32).reshape(8, 128).T.copy(),
        "fc1br": inputs["fc1_b"].astype(f32).reshape(2, 128).T.copy(),
        "fc2br": inputs["fc2_b"].astype(f32).reshape(2, 128).T.copy(),
        "b3rep": np.ascontiguousarray(
            np.broadcast_to(inputs["fc3_b"].astype(f32), (128, Q))),
        "fcqwb": inputs["fcqw_b"].astype(f32).reshape(128, 1),
        "fcqlb": inputs["fcql_b"].astype(f32).reshape(128, 1),
    }
    return shared


def _per_core(inputs, c):
    f32 = np.float32
    sl = slice(c * BS, (c + 1) * BS)

    def fmT(a):  # [BS, 256] -> [2, 128, BS]
        return np.ascontiguousarray(a.T.astype(f32)).reshape(2, 128, BS)

    return {
        "x_fm": fmT(inputs["x"][sl, 0, :]),
        "h1_fm": fmT(inputs["h1"][0, sl]),
        "c1_fm": fmT(inputs["c1"][0, sl]),
        "h2_fm": fmT(inputs["h2"][0, sl]),
        "c2_fm": fmT(inputs["c2"][0, sl]),
    }


def kernel(**inputs):
    key = "nc"
    if key not in _CACHE:
        _CACHE[key] = _build_nc()
    nc = _CACHE[key]

    shared = _prep_shared(inputs)
    in_maps = []
    for c in range(NCORES):
        m = dict(shared)
        m.update(_per_core(inputs, c))
        in_maps.append(m)

    from concourse.bass_utils import run_bass_kernel_spmd
    res = run_bass_kernel_spmd(nc, in_maps, list(range(NCORES)))
    return assemble(res.results)


def assemble(results):
    traj = np.zeros((B, DELTA, K4, 2), np.float32)
    for c, r in enumerate(results):
        idx = r["idx_out"].reshape(2, 128, 20).astype(np.int64)
        for bc in range(2):
            rows = slice(c * BS + bc * 128, c * BS + (bc + 1) * 128)
            top4 = idx[bc, :, 0:4]
            traj[rows, 0, :, 0] = (top4 % QL).astype(np.float32)
            traj[rows, 0, :, 1] = (top4 // QL).astype(np.float32)
            greedy = idx[bc, :, 4:4 + DELTA - 1]
            traj[rows, 1:, 0, 0] = (greedy % QL).astype(np.float32)
            traj[rows, 1:, 0, 1] = (greedy // QL).astype(np.float32)
    return traj


# revision 37
# speedup vs baseline: 1.0531x; 1.0531x over previous
"""Trainium2 Bass kernel for nn_Decoder (2-layer LSTM + 3 FC + top-k beam decode).

Strategy: pure data parallelism over batch (2048 -> 8 cores x 256).
All activations feature-major [feat, batch]. All matmuls in float32r
(FP22 multiply, fp32 accumulate, 1 cycle/row at free-dim >= 256).
Argmax/top-k via DVE max8 + max_index; embedding gather via one-hot matmul.
Host assembles the [B,16,4,2] trajectory from per-step argmax indices.
"""
import numpy as np

B, D, H = 2048, 256, 256
K4, QW, QL, DELTA = 4, 100, 100, 16
Q = QW * QL
NCORES = 8
BS = B // NCORES          # 256 rows per core
NT = 20                   # fc3 N-tiles of 500
TW = 500

_CACHE = {}


def _build_nc(delta=DELTA, dbg=False):
    import concourse.mybir as mybir
    import concourse.tile as tile
    import concourse.bacc as bacc
    from concourse.masks import make_identity

    F32 = mybir.dt.float32
    U32 = mybir.dt.uint32
    AF = mybir.ActivationFunctionType
    ALU = mybir.AluOpType

    nc = bacc.Bacc(None, target_bir_lowering=False, debug=False)

    def din(name, shape, dt=F32):
        return nc.dram_tensor(name, shape, dt, kind="ExternalInput")

    F32R = mybir.dt.float32r
    x_in = din("x_fm", [2, 128, BS], F32R)
    h1_in = din("h1_fm", [2, 128, BS], F32R)
    c1_in = din("c1_fm", [2, 128, BS])
    h2_in = din("h2_fm", [2, 128, BS], F32R)
    c2_in = din("c2_fm", [2, 128, BS])
    w1ih_in = din("w1ihT", [2, 128, 4 * H], F32R)
    w1hh_in = din("w1hhT", [2, 128, 4 * H], F32R)
    w2ih_in = din("w2ihT", [2, 128, 4 * H], F32R)
    w2hh_in = din("w2hhT", [2, 128, 4 * H], F32R)
    fc1_in = din("fc1T", [2, 128, H], F32R)
    fc2_in = din("fc2T", [2, 128, H], F32R)
    fc3_in = din("fc3T", [2, 128, Q], F32R)
    fcqw_in = din("fcqwT", [100, 128], F32R)
    fcql_in = din("fcqlT", [100, 128], F32R)
    b1_in = din("b1r", [128, 8])
    b2_in = din("b2r", [128, 8])
    fc1b_in = din("fc1br", [128, 2])
    fc2b_in = din("fc2br", [128, 2])
    b3rep_in = din("b3rep", [128, Q])
    fcqwb_in = din("fcqwb", [128, 1])
    fcqlb_in = din("fcqlb", [128, 1])

    idx_out = nc.dram_tensor("idx_out", [2, 128, 20], U32, kind="ExternalOutput")

    with tile.TileContext(nc) as tc:
        with (
            tc.tile_pool(name="wp", bufs=1) as wp,
            tc.tile_pool(name="st", bufs=1) as st,
            tc.tile_pool(name="wk", bufs=2) as wk,
            tc.tile_pool(name="ps", bufs=2, space="PSUM") as ps,
        ):
            # ---- load weights / consts ----
            def wload(src, shape, tag, dt=F32):
                t = wp.tile(shape, dt, tag=tag, name=tag)
                if len(shape) == 3 and shape[1] == 2:
                    nc.sync.dma_start(t[:], src[:].rearrange("c p f -> p c f"))
                else:
                    nc.sync.dma_start(t[:], src[:])
                return t

            w1ih = wload(w1ih_in, [128, 2, 4 * H], "w1ih", F32R)
            w1hh = wload(w1hh_in, [128, 2, 4 * H], "w1hh", F32R)
            w2ih = wload(w2ih_in, [128, 2, 4 * H], "w2ih", F32R)
            w2hh = wload(w2hh_in, [128, 2, 4 * H], "w2hh", F32R)
            fc1 = wload(fc1_in, [128, 2, H], "fc1", F32R)
            fc2 = wload(fc2_in, [128, 2, H], "fc2", F32R)
            fc3w = wload(fc3_in, [128, 2, Q], "fc3w", F32R)
            fcqw = wload(fcqw_in, [100, 128], "fcqw", F32R)
            fcql = wload(fcql_in, [100, 128], "fcql", F32R)
            b1r = wload(b1_in, [128, 8], "b1r")
            b2r = wload(b2_in, [128, 8], "b2r")
            fc1b = wload(fc1b_in, [128, 2], "fc1b")
            fc2b = wload(fc2b_in, [128, 2], "fc2b")
            b3rep = wload(b3rep_in, [128, Q], "b3rep")
            fcqwb = wload(fcqwb_in, [128, 1], "fcqwb")
            fcqlb = wload(fcqlb_in, [128, 1], "fcqlb")

            ident = wp.tile([128, 128], F32)
            make_identity(nc, ident[:])
            io_f = wp.tile([128, 100], F32)
            nc.gpsimd.iota(io_f[:], pattern=[[1, 100]], base=0, channel_multiplier=0,
                           allow_small_or_imprecise_dtypes=True)
            io100 = wp.tile([128, 100], F32)
            nc.gpsimd.iota(io100[:], pattern=[[100, 100]], base=0,
                           channel_multiplier=0,
                           allow_small_or_imprecise_dtypes=True)
            io32 = wp.tile([128, 32], F32)
            nc.gpsimd.iota(io32[:], pattern=[[1, 32]], base=0, channel_multiplier=0,
                           allow_small_or_imprecise_dtypes=True)
            qoff = wp.tile([128, 4], F32)
            nc.gpsimd.iota(qoff[:], pattern=[[2500, 4]], base=0,
                           channel_multiplier=0,
                           allow_small_or_imprecise_dtypes=True)

            # ---- persistent states (feature-major [128, chunk, BS]) ----
            def sload(src, tag, dt=F32):
                t = st.tile([128, 2, BS], dt, tag=tag, name=tag)
                nc.sync.dma_start(t[:], src[:].rearrange("c p b -> p c b"))
                return t

            h1_t = sload(h1_in, "h1", F32R)
            c1_t = sload(c1_in, "c1")
            h2_t = sload(h2_in, "h2", F32R)
            c2_t = sload(c2_in, "c2")
            emb_t = st.tile([128, 2, BS], F32R, tag="emb", name="emb")
            nc.sync.dma_start(emb_t[:], x_in[:].rearrange("c p b -> p c b"))
            outi = st.tile([128, 2, 20], U32, tag="outi", name="outi")
            nc.vector.memset(outi[:], 0)

            def pbig():
                return ps.tile([128, 4, 512], F32, tag="big", name="big")

            def lstm_layer(inp, hT, cT, wih, whh, br):
                gpt = pbig()

                def gsl(g):
                    return gpt[:, g // 2, (g % 2) * 256:(g % 2) * 256 + 256]

                for g in range(8):
                    sl = slice(128 * g, 128 * (g + 1))
                    # h-side first: h is ready early, the input side is late
                    nc.tensor.matmul(gsl(g), whh[:, 0, sl], hT[:, 0, :],
                                     start=True, stop=False)
                    nc.tensor.matmul(gsl(g), whh[:, 1, sl], hT[:, 1, :],
                                     start=False, stop=False)
                    nc.tensor.matmul(gsl(g), wih[:, 0, sl], inp[:, 0, :],
                                     start=False, stop=False)
                    nc.tensor.matmul(gsl(g), wih[:, 1, sl], inp[:, 1, :],
                                     start=False, stop=True)
                for ch in range(2):
                    ve = nc.vector
                    si = wk.tile([128, 256], F32, tag="si", bufs=1)
                    sf = wk.tile([128, 256], F32, tag="sf", bufs=1)
                    tg = wk.tile([128, 256], F32, tag="tg", bufs=1)
                    so = wk.tile([128, 256], F32, tag="so", bufs=1)
                    nc.scalar.activation(si[:], gsl(0 + ch), AF.Sigmoid,
                                         bias=br[:, 0 + ch:1 + ch])
                    nc.scalar.activation(sf[:], gsl(2 + ch), AF.Sigmoid,
                                         bias=br[:, 2 + ch:3 + ch])
                    nc.scalar.activation(tg[:], gsl(4 + ch), AF.Tanh,
                                         bias=br[:, 4 + ch:5 + ch])
                    nc.scalar.activation(so[:], gsl(6 + ch), AF.Sigmoid,
                                         bias=br[:, 6 + ch:7 + ch])
                    t1 = wk.tile([128, 256], F32, tag="t1", bufs=1)
                    t2 = wk.tile([128, 256], F32, tag="t2", bufs=1)
                    ve.tensor_mul(t1[:], sf[:], cT[:, ch, :])
                    ve.tensor_mul(t2[:], si[:], tg[:])
                    ve.tensor_add(cT[:, ch, :], t1[:], t2[:])
                    t3 = wk.tile([128, 256], F32, tag="t3", bufs=1)
                    nc.scalar.activation(t3[:], cT[:, ch, :], AF.Tanh)
                    ve.tensor_mul(hT[:, ch, :], so[:], t3[:])

            for t in range(delta):
                inp = emb_t
                lstm_layer(inp, h1_t, c1_t, w1ih, w1hh, b1r)
                lstm_layer(h1_t, h2_t, c2_t, w2ih, w2hh, b2r)

                # fc1, fc2 (feature-major out)
                y1 = st.tile([128, 2, BS], F32R, tag="y1")
                y2 = st.tile([128, 2, BS], F32R, tag="y2")
                for (dst, w, bb, src) in ((y1, fc1, fc1b, h2_t), (y2, fc2, fc2b, y1)):
                    fpt = pbig()
                    for m in range(2):
                        fsl = fpt[:, m // 2, (m % 2) * 256:(m % 2) * 256 + 256]
                        sl = slice(128 * m, 128 * (m + 1))
                        nc.tensor.matmul(fsl, w[:, 0, sl], src[:, 0, :],
                                         start=True, stop=False)
                        nc.tensor.matmul(fsl, w[:, 1, sl], src[:, 1, :],
                                         start=False, stop=True)
                        nc.scalar.activation(dst[:, m, :], fsl, AF.Identity,
                                             bias=bb[:, m:m + 1])

                # fc3 per batch-chunk: 5 groups x 4 tiles of 500
                ohwT = wk.tile([100, 256], F32R, tag="ohwT", name="ohwT")
                ohlT = wk.tile([100, 256], F32R, tag="ohlT", name="ohlT")
                for bc in range(2):
                    bsl = slice(128 * bc, 128 * (bc + 1))
                    lq = [wk.tile([128, 2500], F32, tag="logq", name="logq",
                                  bufs=2) for _ in range(4)]
                    nscan = [0]
                    if t == 0:
                        cand_v = wk.tile([128, 32], F32, tag="candv", name="candv")
                        cand_i = wk.tile([128, 32], F32, tag="candi", name="candi")

                        def scan_ready(upto):
                            # top-8 per quarter (beam-4 output needs them)
                            while nscan[0] < 4 and (nscan[0] + 1) * 2500 <= upto:
                                qt = nscan[0]
                                m8q = wk.tile([128, 8], F32, tag="m8q", name="m8q")
                                i8q = wk.tile([128, 8], U32, tag="i8q", name="i8q")
                                nc.vector.max(m8q[:], lq[qt][:])
                                nc.vector.max_index(i8q[:], m8q[:], lq[qt][:])
                                nc.vector.tensor_copy(cand_v[:, 8 * qt:8 * qt + 8],
                                                      m8q[:])
                                i8f = wk.tile([128, 8], F32, tag="i8f", name="i8f")
                                nc.vector.tensor_copy(i8f[:], i8q[:])
                                nc.vector.tensor_scalar(
                                    cand_i[:, 8 * qt:8 * qt + 8], i8f[:],
                                    float(2500 * qt), None, op0=ALU.add)
                                nscan[0] += 1
                    else:
                        # greedy steps: GPSIMD pre-maxes each quarter 2500->625,
                        # DVE scans only the pre-maxed array for the value and
                        # the full quarter once for the index.
                        qv = wk.tile([128, 8], F32, tag="qv", name="qv", bufs=1)
                        nc.vector.memset(qv[:], -3.0e38)
                        iq4 = wk.tile([128, 4], F32, tag="iq4", name="iq4",
                                      bufs=1)

                        def scan_ready(upto):
                            while nscan[0] < 4 and (nscan[0] + 1) * 2500 <= upto:
                                qt = nscan[0]
                                m8q = wk.tile([128, 8], F32, tag="m8q",
                                              name="m8q")
                                nc.vector.max(m8q[:], lq[qt][:])
                                nc.vector.tensor_copy(qv[:, qt:qt + 1],
                                                      m8q[:, 0:1])
                                i8q = wk.tile([128, 8], U32, tag="i8q",
                                              name="i8q")
                                nc.vector.max_index(i8q[:], m8q[:], lq[qt][:])
                                nc.vector.tensor_copy(iq4[:, qt:qt + 1],
                                                      i8q[:, 0:1])
                                nscan[0] += 1

                    for grp in range(5):
                        gp3 = pbig()
                        for tt in range(4):
                            n0 = (grp * 4 + tt) * TW
                            o = gp3[:, tt, 0:TW]
                            nc.tensor.matmul(o, y2[:, 0, bsl],
                                             fc3w[:, 0, n0:n0 + TW],
                                             start=True, stop=False)
                            nc.tensor.matmul(o, y2[:, 1, bsl],
                                             fc3w[:, 1, n0:n0 + TW],
                                             start=False, stop=True)
                        # evacuate per psum tile (+bias) into quarter tiles;
                        # two tiles per group go via ACT copy + GPSIMD add to
                        # relieve the vector engine
                        for tt in range(4):
                            n0 = (grp * 4 + tt) * TW
                            qt = n0 // 2500
                            dst = lq[qt][:, n0 - 2500 * qt:n0 - 2500 * qt + TW]
                            if tt >= 2 and grp < 4:
                                raw = wk.tile([128, TW], F32, tag="raw",
                                              name="raw", bufs=1)
                                nc.scalar.copy(raw[:], gp3[:, tt, 0:TW])
                                nc.gpsimd.tensor_add(dst, raw[:],
                                                     b3rep[:, n0:n0 + TW])
                            else:
                                nc.vector.tensor_add(dst, gp3[:, tt, 0:TW],
                                                     b3rep[:, n0:n0 + TW])
                        scan_ready(grp * 2000 + 2000)

                    qf = wk.tile([128, 1], F32, tag="qf", name="qf")
                    if t == 0:
                        # merge 32 candidates -> top-4 beams
                        vm8 = wk.tile([128, 8], F32, tag="vm8", name="vm8")
                        pm8 = wk.tile([128, 8], U32, tag="pm8", name="pm8")
                        nc.vector.max(vm8[:], cand_v[:])
                        nc.vector.max_index(pm8[:], vm8[:], cand_v[:])
                        pmf = wk.tile([128, 8], F32, tag="pmf", name="pmf")
                        nc.vector.tensor_copy(pmf[:], pm8[:])
                        qsel = wk.tile([128, 4], F32, tag="qsel", name="qsel")
                        for kk in range(4):
                            ohp = wk.tile([128, 32], F32, tag="ohp", name="ohp")
                            nc.vector.tensor_scalar(ohp[:], io32[:],
                                                    pmf[:, kk:kk + 1],
                                                    None, op0=ALU.is_equal)
                            tmq = wk.tile([128, 32], F32, tag="tmq", name="tmq")
                            nc.vector.tensor_mul(tmq[:], ohp[:], cand_i[:])
                            nc.vector.tensor_reduce(qsel[:, kk:kk + 1], tmq[:],
                                                    axis=mybir.AxisListType.X,
                                                    op=ALU.add)
                        nc.vector.tensor_copy(outi[:, bc, 0:4], qsel[:, 0:4])
                        nc.vector.tensor_copy(qf[:], qsel[:, 0:1])
                    else:
                        # pick winning quarter, add its base offset
                        vm8 = wk.tile([128, 8], F32, tag="vm8", name="vm8")
                        nc.vector.max(vm8[:], qv[:])
                        oh4 = wk.tile([128, 4], F32, tag="oh4", name="oh4",
                                      bufs=1)
                        nc.vector.tensor_scalar(oh4[:], qv[:, 0:4],
                                                vm8[:, 0:1], None,
                                                op0=ALU.is_equal)
                        nc.vector.tensor_add(iq4[:], iq4[:], qoff[:])
                        nc.vector.tensor_mul(oh4[:], oh4[:], iq4[:])
                        nc.vector.tensor_reduce(qf[:], oh4[:],
                                                axis=mybir.AxisListType.X,
                                                op=ALU.add)
                        nc.vector.tensor_copy(outi[:, bc, 4 + t - 1:5 + t - 1],
                                              qf[:])
                    if t == delta - 1:
                        continue
                    # ohw[b,j] = (100j <= q) & (100j > q-100)
                    m_ge = wk.tile([128, 100], F32, tag="mge", name="mge", bufs=1)
                    nc.vector.tensor_scalar(m_ge[:], io100[:], qf[:], None,
                                            op0=ALU.is_le)
                    qm = wk.tile([128, 1], F32, tag="qm", name="qm")
                    nc.vector.tensor_scalar(qm[:], qf[:], -100.0, None, op0=ALU.add)
                    m_lt = wk.tile([128, 100], F32, tag="mlt", name="mlt", bufs=1)
                    nc.vector.tensor_scalar(m_lt[:], io100[:], qm[:], None,
                                            op0=ALU.is_gt)
                    ohw = wk.tile([128, 100], F32, tag="ohw", name="ohw", bufs=1)
                    nc.vector.tensor_mul(ohw[:], m_ge[:], m_lt[:])
                    tm = wk.tile([128, 100], F32, tag="tm", name="tm", bufs=1)
                    nc.vector.tensor_mul(tm[:], ohw[:], io_f[:])
                    fwf = wk.tile([128, 1], F32, tag="fwf", name="fwf")
                    nc.vector.tensor_reduce(fwf[:], tm[:], axis=mybir.AxisListType.X,
                                            op=ALU.add)
                    flf = wk.tile([128, 1], F32, tag="flf", name="flf")
                    nc.vector.tensor_scalar(flf[:], fwf[:], -100.0, qf[:],
                                            op0=ALU.mult, op1=ALU.add)
                    ohl = wk.tile([128, 100], F32, tag="ohl", name="ohl", bufs=1)
                    nc.vector.tensor_scalar(ohl[:], io_f[:], flf[:], None,
                                            op0=ALU.is_equal)
                    ptr = pbig()
                    pw = ptr[0:100, 0, 0:128]
                    nc.tensor.transpose(pw, ohw[:], ident[:])
                    nc.vector.tensor_copy(ohwT[:, bsl128(bc)], pw)
                    pl = ptr[0:100, 1, 0:128]
                    nc.tensor.transpose(pl, ohl[:], ident[:])
                    nc.vector.tensor_copy(ohlT[:, bsl128(bc)], pl)

                if t == delta - 1:
                    continue
                # embedding gather matmuls + bias
                pet = pbig()
                pe0 = pet[:, 0, 0:BS]
                pe1 = pet[:, 1, 0:BS]
                nc.tensor.matmul(pe0, fcqw[:], ohwT[:], start=True, stop=True)
                nc.tensor.matmul(pe1, fcql[:], ohlT[:], start=True, stop=True)
                nc.scalar.activation(emb_t[:, 0, :], pe0, AF.Identity,
                                     bias=fcqwb[:])
                nc.scalar.activation(emb_t[:, 1, :], pe1, AF.Identity,
                                     bias=fcqlb[:])

            for bc in range(2):
                nc.sync.dma_start(idx_out[bc], outi[:, bc, :])
    nc.finalize()
    return nc


def bsl128(bc):
    return slice(128 * bc, 128 * (bc + 1))


def _prep_shared(inputs):
    f32 = np.float32

    def fm(w):  # [out,in] -> lhsT layout [2,128,out]
        wt = np.ascontiguousarray(w.T.astype(f32))        # [in, out]
        return wt.reshape(2, 128, wt.shape[1])

    fc3T = np.ascontiguousarray(inputs["fc3_W"].T.astype(f32))  # [256, 10000]

    shared = {
        "w1ihT": fm(inputs["lstm1_Wih"]),
        "w1hhT": fm(inputs["lstm1_Whh"]),
        "w2ihT": fm(inputs["lstm2_Wih"]),
        "w2hhT": fm(inputs["lstm2_Whh"]),
        "fc1T": fm(inputs["fc1_W"]),
        "fc2T": fm(inputs["fc2_W"]),
        "fc3T": fc3T.reshape(2, 128, Q),
        "fcqwT": np.ascontiguousarray(inputs["fcqw_W"].T.astype(f32))[:, :],
        "fcqlT": np.ascontiguousarray(inputs["fcql_W"].T.astype(f32))[:, :],
        "b1r": inputs["lstm1_b"].astype(f32).reshape(8, 128).T.copy(),
        "b2r": inputs["lstm2_b"].astype(f32).reshape(8, 128).T.copy(),
        "fc1br": inputs["fc1_b"].astype(f32).reshape(2, 128).T.copy(),
        "fc2br": inputs["fc2_b"].astype(f32).reshape(2, 128).T.copy(),
        "b3rep": np.ascontiguousarray(
            np.broadcast_to(inputs["fc3_b"].astype(f32), (128, Q))),
        "fcqwb": inputs["fcqw_b"].astype(f32).reshape(128, 1),
        "fcqlb": inputs["fcql_b"].astype(f32).reshape(128, 1),
    }
    return shared


def _per_core(inputs, c):
    f32 = np.float32
    sl = slice(c * BS, (c + 1) * BS)

    def fmT(a):  # [BS, 256] -> [2, 128, BS]
        return np.ascontiguousarray(a.T.astype(f32)).reshape(2, 128, BS)

    return {
        "x_fm": fmT(inputs["x"][sl, 0, :]),
        "h1_fm": fmT(inputs["h1"][0, sl]),
        "c1_fm": fmT(inputs["c1"][0, sl]),
        "h2_fm": fmT(inputs["h2"][0, sl]),
        "c2_fm": fmT(inputs["c2"][0, sl]),
    }


def kernel(**inputs):
    key = "nc"
    if key not in _CACHE:
        _CACHE[key] = _build_nc()
    nc = _CACHE[key]

    shared = _prep_shared(inputs)
    in_maps = []
    for c in range(NCORES):
        m = dict(shared)
        m.update(_per_core(inputs, c))
        in_maps.append(m)

    from concourse.bass_utils import run_bass_kernel_spmd
    res = run_bass_kernel_spmd(nc, in_maps, list(range(NCORES)))
    return assemble(res.results)


def assemble(results):
    traj = np.zeros((B, DELTA, K4, 2), np.float32)
    for c, r in enumerate(results):
        idx = r["idx_out"].reshape(2, 128, 20).astype(np.int64)
        for bc in range(2):
            rows = slice(c * BS + bc * 128, c * BS + (bc + 1) * 128)
            top4 = idx[bc, :, 0:4]
            traj[rows, 0, :, 0] = (top4 % QL).astype(np.float32)
            traj[rows, 0, :, 1] = (top4 // QL).astype(np.float32)
            greedy = idx[bc, :, 4:4 + DELTA - 1]
            traj[rows, 1:, 0, 0] = (greedy % QL).astype(np.float32)
            traj[rows, 1:, 0, 1] = (greedy // QL).astype(np.float32)
    return traj
